# revision 3
# baseline (speedup 1.0000x reference)
"""Space-to-depth (8x8 chessboard) kernel for Trainium2.

Full input  : (32, 256, 256, 32) f32
Full output : (32, 8, 8, 32768) f32
out[b, i, j] = inputs[b, i*32:(i+1)*32, j*32:(j+1)*32, :].reshape(-1)

Sharding: batch dim (32) split across 8 NeuronCores (pure data parallel,
no communication) -> 4 examples per core.

Per core the op is pure HBM->HBM data movement, done entirely with DMA
access patterns (no compute engines). Key layout fact: within one
(example b, 32-row band i), iterating (r, j, elem) makes the source AP
contiguous and the destination a 3D AP, so a single DMA moves a
half-band (16 rows = 512 KiB) in 4 KiB contiguous chunks:

  src [[8192, 16], [1024, 8], [1, 1024]]   (contiguous 32 KiB per row r)
  dst [[1024, 16], [32768, 8], [1, 1024]]  (4 KiB chunks, 32 KiB stride)

Performance notes (measured on trn2 via NTFF traces):
- SDMA engine assignment is (outer AP dim index) mod 16, so outer count
  16 engages all 16 SDMA engines.
- Keep HWDGE DMAs at <= 128 descriptors (outer <= 16): larger DMAs
  hit a slow descriptor-generation fallback.
- Each 4 KiB descriptor streams at ~25.9 GB/s (158 ns); descriptors on
  one engine's ring run back-to-back (median start-to-start 156 ns).
- Issuing from both HWDGE queues (sync=SP + scalar=ACT) beats one queue.
- Per-instruction `then_inc` semaphore descriptors stall the engine
  ~300-1500 ns each at every instruction boundary: the sem descriptor
  cannot fire until the engine's last data write is receipt-confirmed in
  HBM, and descriptors are processed in ring order. Dropping the sem
  from all but the final instruction per queue (FIFO per ring makes the
  last one a fence) removes ~26 us of stalls per engine.
"""

import numpy as np

_B_PER_CORE = 4
_N_CORES = 8
_IN_SHAPE = (_B_PER_CORE, 256, 256, 32)
_OUT_SHAPE = (_B_PER_CORE, 8, 8, 32768)
_EX = 256 * 256 * 32      # elements per example  (2097152)
_BAND = 32 * 256 * 32     # elements per (example, row-band)  (262144)

_CACHE = {}


def build_nc():
    import concourse.bass as bass
    import concourse.mybir as mybir

    nc = bass.Bass(target_bir_lowering=False)
    x = nc.dram_tensor("x", list(_IN_SHAPE), mybir.dt.float32, kind="ExternalInput")
    y = nc.dram_tensor("y", list(_OUT_SHAPE), mybir.dt.float32, kind="ExternalOutput")

    # 64 half-band jobs: (example b, band i, first row r0), 16 rows each.
    jobs = [
        (b, i, h * 16)
        for b in range(_B_PER_CORE)
        for i in range(8)
        for h in range(2)
    ]
    # Three descriptor-generation streams: the two HWDGE rings (SP + ACT)
    # plus the gpsimd SWDGE ring, so generation stays ahead of engine drain.
    gp_jobs = [j for k, j in enumerate(jobs) if k % 5 == 4]
    hw_jobs = [j for k, j in enumerate(jobs) if k % 5 != 4]

    def issue(engine, my_jobs, sem):
        n = 0
        for b, i, r0 in my_jobs:
            off = b * _EX + i * _BAND
            src = bass.AP(x, off + r0 * 8192, [[8192, 16], [1024, 8], [1, 1024]])
            dst = bass.AP(y, off + r0 * 1024, [[1024, 16], [32768, 8], [1, 1024]])
            engine.dma_start(out=dst, in_=src).then_inc(sem, 16)
            n += 16
        if n:
            engine.wait_ge(sem, n)

    with (
        nc.semaphore("sp_sem") as sp_sem,
        nc.semaphore("act_sem") as act_sem,
        nc.semaphore("gp_sem") as gp_sem,
        nc.Block(no_gpsimd_drain=True) as block,
    ):

        @block.sync
        def _(sync):
            issue(sync, hw_jobs[0::2], sp_sem)

        @block.scalar
        def _(scalar):
            issue(scalar, hw_jobs[1::2], act_sem)

        @block.gpsimd
        def _(gpsimd):
            issue(gpsimd, gp_jobs, gp_sem)

    return nc


def _get_nc():
    if "nc" not in _CACHE:
        _CACHE["nc"] = build_nc()
    return _CACHE["nc"]


def kernel(inputs: np.ndarray) -> np.ndarray:
    from concourse.bass_utils import run_bass_kernel_spmd

    inputs = np.ascontiguousarray(np.asarray(inputs, dtype=np.float32))
    assert inputs.shape == (_B_PER_CORE * _N_CORES,) + _IN_SHAPE[1:]

    nc = _get_nc()
    in_maps = [
        {"x": np.ascontiguousarray(inputs[c * _B_PER_CORE : (c + 1) * _B_PER_CORE])}
        for c in range(_N_CORES)
    ]
    res = run_bass_kernel_spmd(nc, in_maps, core_ids=list(range(_N_CORES)))
    return np.concatenate([r["y"] for r in res.results], axis=0)


# revision 4
# speedup vs baseline: 1.0306x; 1.0306x over previous
"""Space-to-depth (8x8 chessboard) kernel for Trainium2.

Full input  : (32, 256, 256, 32) f32
Full output : (32, 8, 8, 32768) f32
out[b, i, j] = inputs[b, i*32:(i+1)*32, j*32:(j+1)*32, :].reshape(-1)

Sharding: batch dim (32) split across 8 NeuronCores (pure data parallel,
no communication) -> 4 examples per core.

Per core the op is pure HBM->HBM data movement, done entirely with DMA
access patterns (no compute engines). Key layout fact: within one
(example b, 32-row band i), iterating (r, j, elem) makes the source AP
contiguous and the destination a 3D AP, so a single DMA moves a
half-band (16 rows = 512 KiB) in 4 KiB contiguous chunks:

  src [[8192, 16], [1024, 8], [1, 1024]]   (contiguous 32 KiB per row r)
  dst [[1024, 16], [32768, 8], [1, 1024]]  (4 KiB chunks, 32 KiB stride)

Performance notes (measured on trn2 via NTFF traces):
- SDMA engine assignment is (outer AP dim index) mod 16, so outer count
  16 engages all 16 SDMA engines.
- Keep HWDGE DMAs at <= 128 descriptors (outer <= 16): larger DMAs
  hit a slow descriptor-generation fallback.
- Each 4 KiB descriptor streams at ~25.9 GB/s (158 ns); descriptors on
  one engine's ring run back-to-back (median start-to-start 156 ns).
- Issuing from both HWDGE queues (sync=SP + scalar=ACT) beats one queue.
- Per-instruction `then_inc` semaphore descriptors stall the engine
  ~300-1500 ns each at every instruction boundary: the sem descriptor
  cannot fire until the engine's last data write is receipt-confirmed in
  HBM, and descriptors are processed in ring order. Dropping the sem
  from all but the final instruction per queue (FIFO per ring makes the
  last one a fence) removes ~26 us of stalls per engine.
"""

import numpy as np

_B_PER_CORE = 4
_N_CORES = 8
_IN_SHAPE = (_B_PER_CORE, 256, 256, 32)
_OUT_SHAPE = (_B_PER_CORE, 8, 8, 32768)
_EX = 256 * 256 * 32      # elements per example  (2097152)
_BAND = 32 * 256 * 32     # elements per (example, row-band)  (262144)

_CACHE = {}


def build_nc():
    import concourse.bass as bass
    import concourse.mybir as mybir

    nc = bass.Bass(target_bir_lowering=False)
    x = nc.dram_tensor("x", list(_IN_SHAPE), mybir.dt.float32, kind="ExternalInput")
    y = nc.dram_tensor("y", list(_OUT_SHAPE), mybir.dt.float32, kind="ExternalOutput")

    # 32 full-band jobs: (example b, band i), 32 rows each, 256 descs/side.
    # SWDGE (gpsimd) generates descriptors at ~1 us + 0.34 ns/desc per
    # instruction (~1.2 us for a band that takes ~2.5 us to drain), so a
    # single SWDGE stream keeps all 16 engines fed with 2x slack, unlike
    # the two HWDGE rings which cap at 128 descs (outer<=16) and generate
    # at ~1 instr/0.95 us vs drain 1/1.26 us (barely ahead -> bubbles).
    jobs = [(b, i) for b in range(_B_PER_CORE) for i in range(8)]

    def issue(engine, my_jobs, sem):
        n = 0
        for b, i in my_jobs:
            off = b * _EX + i * _BAND
            src = bass.AP(x, off, [[8192, 32], [1024, 8], [1, 1024]])
            dst = bass.AP(y, off, [[1024, 32], [32768, 8], [1, 1024]])
            engine.dma_start(out=dst, in_=src).then_inc(sem, 16)
            n += 16
        if n:
            engine.wait_ge(sem, n)

    with (
        nc.semaphore("gp_sem") as gp_sem,
        nc.Block(no_gpsimd_drain=True) as block,
    ):

        @block.gpsimd
        def _(gpsimd):
            issue(gpsimd, jobs, gp_sem)

    return nc


def _get_nc():
    if "nc" not in _CACHE:
        _CACHE["nc"] = build_nc()
    return _CACHE["nc"]


def kernel(inputs: np.ndarray) -> np.ndarray:
    from concourse.bass_utils import run_bass_kernel_spmd

    inputs = np.ascontiguousarray(np.asarray(inputs, dtype=np.float32))
    assert inputs.shape == (_B_PER_CORE * _N_CORES,) + _IN_SHAPE[1:]

    nc = _get_nc()
    in_maps = [
        {"x": np.ascontiguousarray(inputs[c * _B_PER_CORE : (c + 1) * _B_PER_CORE])}
        for c in range(_N_CORES)
    ]
    res = run_bass_kernel_spmd(nc, in_maps, core_ids=list(range(_N_CORES)))
    return np.concatenate([r["y"] for r in res.results], axis=0)


# revision 5
# speedup vs baseline: 1.1820x; 1.1470x over previous
"""Space-to-depth (8x8 chessboard) kernel for Trainium2.

Full input  : (32, 256, 256, 32) f32
Full output : (32, 8, 8, 32768) f32
out[b, i, j] = inputs[b, i*32:(i+1)*32, j*32:(j+1)*32, :].reshape(-1)

Sharding: batch dim (32) split across 8 NeuronCores (pure data parallel,
no communication) -> 4 examples per core.

Pure HBM->HBM data movement via DMA access patterns. See kernel_base.py
docstring for the measured baseline facts. This variant decorrelates the
two HWDGE queues' address streams: queue SP works examples 0-1 while
queue ACT works examples 2-3, so the 32 concurrent per-engine streams
touch distant HBM regions instead of marching through the same band.
"""

import numpy as np

_B_PER_CORE = 4
_N_CORES = 8
_IN_SHAPE = (_B_PER_CORE, 256, 256, 32)
_OUT_SHAPE = (_B_PER_CORE, 8, 8, 32768)
_EX = 256 * 256 * 32      # elements per example  (2097152)
_BAND = 32 * 256 * 32     # elements per (example, row-band)  (262144)

_CACHE = {}


def build_nc():
    import concourse.bass as bass
    import concourse.mybir as mybir

    nc = bass.Bass(target_bir_lowering=False)
    x = nc.dram_tensor("x", list(_IN_SHAPE), mybir.dt.float32, kind="ExternalInput")
    y = nc.dram_tensor("y", list(_OUT_SHAPE), mybir.dt.float32, kind="ExternalOutput")

    # Engine-15 skew (it is ~1.25x slower): the b<2 second half-bands carry
    # 15 rows; their skipped row 31 is covered by two outer-8 orphan DMAs
    # (engines 0-7 only).  All skewed jobs are b<2 and live on queue SP;
    # queue ACT gets the b>=2 examples, decorrelating the address streams.
    q1_jobs = [
        (b, i, h * 16, 15 if h == 1 else 16)
        for b in range(2)
        for i in range(8)
        for h in range(2)
    ] + [("orph", 0), ("orph", 1)]
    q2_jobs = [
        (b, i, h * 16, 16)
        for b in range(2, 4)
        for i in range(8)
        for h in range(2)
    ]

    def issue(engine, my_jobs, sem):
        n = 0
        for job in my_jobs:
            if job[0] == "orph":
                _, b = job
                src = bass.AP(
                    x, b * _EX + 31 * 8192, [[262144, 8], [1024, 8], [1, 1024]]
                )
                dst = bass.AP(
                    y, b * _EX + 31 * 1024, [[262144, 8], [32768, 8], [1, 1024]]
                )
            else:
                b, i, r0, nr = job
                off = b * _EX + i * _BAND
                src = bass.AP(
                    x, off + r0 * 8192, [[8192, nr], [1024, 8], [1, 1024]]
                )
                dst = bass.AP(
                    y, off + r0 * 1024, [[1024, nr], [32768, 8], [1, 1024]]
                )
            engine.dma_start(out=dst, in_=src).then_inc(sem, 16)
            n += 16
        if n:
            engine.wait_ge(sem, n)

    with (
        nc.semaphore("sp_sem") as sp_sem,
        nc.semaphore("act_sem") as act_sem,
        nc.Block(no_gpsimd_drain=True) as block,
    ):

        @block.sync
        def _(sync):
            issue(sync, q1_jobs, sp_sem)

        @block.scalar
        def _(scalar):
            issue(scalar, q2_jobs, act_sem)

    return nc


def _get_nc():
    if "nc" not in _CACHE:
        _CACHE["nc"] = build_nc()
    return _CACHE["nc"]


def kernel(inputs: np.ndarray) -> np.ndarray:
    from concourse.bass_utils import run_bass_kernel_spmd

    inputs = np.ascontiguousarray(np.asarray(inputs, dtype=np.float32))
    assert inputs.shape == (_B_PER_CORE * _N_CORES,) + _IN_SHAPE[1:]

    nc = _get_nc()
    in_maps = [
        {"x": np.ascontiguousarray(inputs[c * _B_PER_CORE : (c + 1) * _B_PER_CORE])}
        for c in range(_N_CORES)
    ]
    res = run_bass_kernel_spmd(nc, in_maps, core_ids=list(range(_N_CORES)))
    return np.concatenate([r["y"] for r in res.results], axis=0)
